# revision 10
# baseline (speedup 1.0000x reference)
"""Haar DWT pooling (NHWC, 2x2 blocks, all 4 components channel-interleaved).

Full input x: (8, 512, 512, 64) f32 -> output (8, 256, 256, 256) f32.
Sharding: data-parallel over batch; core b handles x[b] (no communication).

Per-core dataflow (x_b: (512,512,64) -> y_b: (256,256,256)):
  - partition p = half*64 + q: half = output_row % 2, q covers input columns
    [8q, 8q+8) (= 4 output pixels). DMA runs: 2KB contiguous per input row on
    load, 4KB per output row on store (the kernel is HBM-bound; ~1-2KB runs
    measured fastest through the SDMA engines, very large runs run slower).
  - loop over chunks of K=16 input rows (32 chunks):
      load   X[128, 4096]  <- x[h0:h0+K]        (2KB runs, SP HWDGE ring)
      DVE    s = r0+r1 (ST tile), d = r0-r1 (in place over r1)
      DVE    four butterfly ops -> comp-planar scratch O2 (dense writes;
             stride-4 DVE writes would run at half rate)
      ACT    OT[c*4+comp] = 0.5 * O2[comp]      (scale + channel interleave
             on the otherwise idle ACT engine, one op per component)
      store  OT -> y[i0:i0+8]                   (4KB runs, ACT HWDGE ring --
             separate ring from loads to avoid head-of-line blocking)

fp32 tensor_tensor on DVE is capped at 1 elem/lane/cycle (no 2x uop), so DVE
instruction count and AP shapes are chosen to keep DVE ~line-rate; DVE (~300us)
and ACT (~270us) then hide under the ~380us HBM stream time.
"""

import numpy as np

import concourse.bacc as bacc
import concourse.mybir as mybir
from concourse.bass_utils import run_bass_kernel_spmd
from concourse.tile import TileContext

N_CORES = 8
H = 512
W = 512
C = 64
P = 128
HF = 2          # output-row stripe factor across partition halves
Q = 64          # w-blocks per row; each block = 8 input cols = 2KB
ROWS_PER_CHUNK = 16


def build_dwt_body(nc, tc, x_ap, out_ap, io_bufs=3, mid_bufs=2):
    """Emit the per-core DWT pooling kernel body under an open TileContext.

    x_ap:   DRAM AP, shape (H, W, C) f32, H divisible by 16
    out_ap: DRAM AP, shape (H//2, W//2, 4*C) f32
    """
    K = ROWS_PER_CHUNK
    h_total = x_ap.shape[0]
    assert x_ap.shape == (h_total, W, C)
    assert out_ap.shape == (h_total // 2, W // 2, 4 * C)
    assert h_total % K == 0
    n_chunks = h_total // K
    MP = K // 4  # row pairs per partition per chunk (= 4)

    dt = mybir.dt.float32
    with (
        tc.tile_pool(name="io", bufs=io_bufs) as io_pool,
        tc.tile_pool(name="mid", bufs=mid_bufs) as mid_pool,
    ):
        for ci in range(n_chunks):
            h0 = ci * K
            i0 = ci * (K // 2)

            # ---- load. Input row h0 + 4m + 2*half + k2 lands in partitions
            #      [64*half, 64*half+64) at free slot (m, k2); 2KB runs.
            #      One DMA per (half, k2): DMA APs allow at most 3 dims.
            xt = io_pool.tile([P, K * 256], dt)
            x_view = x_ap[h0 : h0 + K].rearrange(
                "(m half k2) (q f) c -> half k2 q m (f c)", half=HF, k2=2, q=Q
            )
            for half in range(HF):
                xt_h = xt[64 * half : 64 * (half + 1)].rearrange(
                    "q (m k2 f) -> k2 q m f", k2=2, f=512
                )
                for k2 in range(2):
                    nc.sync.dma_start(out=xt_h[k2], in_=x_view[half, k2])

            # per-partition X layout: (m, k2, wc) with wc = (jl, wp, c)
            xr = xt[:].rearrange("p (m k2 wc) -> p m k2 wc", k2=2, wc=512)
            r0 = xr[:, :, 0, :]  # rows 2i   : (a | b) interleaved over wp
            r1 = xr[:, :, 1, :]  # rows 2i+1 : (c | d)

            # ---- stage 1: vertical butterfly over all columns at once
            st = mid_pool.tile([P, MP * 512], dt)
            sv = st[:].rearrange("p (m wc) -> p m wc", wc=512)
            nc.vector.tensor_add(sv, r0, r1)   # s = r0 + r1
            nc.vector.tensor_sub(r1, r0, r1)   # d = r0 - r1, in place over r1

            # even/odd column views: (m, jl, c)
            s_ = st[:].rearrange("p (m jl wp c) -> p m jl wp c", jl=4, wp=2, c=C)
            d_ = xt[:].rearrange(
                "p (m k2 jl wp c) -> p m k2 jl wp c", k2=2, jl=4, wp=2, c=C
            )
            s0 = s_[:, :, :, 0, :]
            s1 = s_[:, :, :, 1, :]
            d0 = d_[:, :, 1, :, 0, :]
            d1 = d_[:, :, 1, :, 1, :]

            # ---- stage 2: horizontal butterfly into comp-planar scratch
            o2 = mid_pool.tile([P, 4 * MP * 256], dt)
            o2v = o2[:].rearrange("p (comp m jl c) -> p comp m jl c",
                                  comp=4, jl=4, c=C)
            nc.vector.tensor_add(o2v[:, 0], s0, s1)  # LL = s0+s1
            nc.vector.tensor_add(o2v[:, 1], d0, d1)  # LH = d0+d1
            nc.vector.tensor_sub(o2v[:, 2], s0, s1)  # HL = s0-s1
            nc.vector.tensor_sub(o2v[:, 3], d0, d1)  # HH = d0-d1

            # ---- scale by 0.5 + channel interleave on the idle ACT engine
            ot = io_pool.tile([P, MP * 1024], dt)
            ov = ot[:].rearrange("p (m jl c comp) -> p m jl c comp",
                                 jl=4, c=C, comp=4)
            for comp in range(4):
                nc.scalar.mul(ov[:, :, :, :, comp], o2v[:, comp], 0.5)

            # ---- store. Partition (half, q) holds output rows i0+2m+half,
            #      pixels [4q, 4q+4); 4KB runs. One DMA per half, ACT ring.
            o_view = out_ap[i0 : i0 + K // 2].rearrange(
                "(m half) (q px) c -> half q m (px c)", half=HF, q=Q, px=4
            )
            for half in range(HF):
                nc.scalar.dma_start(
                    out=o_view[half],
                    in_=ot[64 * half : 64 * (half + 1)].rearrange(
                        "q (m f) -> q m f", f=1024
                    ),
                )


def build_bass(h=H, io_bufs=3, mid_bufs=2):
    nc = bacc.Bacc(trn_type="TRN2", target_bir_lowering=False, debug=False)
    x_d = nc.dram_tensor("x", [h, W, C], mybir.dt.float32, kind="ExternalInput")
    out_d = nc.dram_tensor(
        "out", [h // 2, W // 2, 4 * C], mybir.dt.float32, kind="ExternalOutput"
    )
    with TileContext(nc) as tc:
        build_dwt_body(nc, tc, x_d.ap(), out_d.ap(), io_bufs=io_bufs, mid_bufs=mid_bufs)
    nc.finalize()
    return nc


_NC_CACHE = {}


def _get_nc():
    if "nc" not in _NC_CACHE:
        _NC_CACHE["nc"] = build_bass()
    return _NC_CACHE["nc"]


def run_spmd(x, **kwargs):
    """Run the 8-core SPMD kernel on full input x (8,512,512,64).

    Returns (output (8,256,256,256) f32, BassKernelResults)."""
    x = np.asarray(x)
    assert x.shape == (N_CORES, H, W, C) and x.dtype == np.float32
    nc = _get_nc()
    in_maps = [{"x": np.ascontiguousarray(x[b])} for b in range(N_CORES)]
    res = run_bass_kernel_spmd(nc, in_maps, core_ids=list(range(N_CORES)), **kwargs)
    out = np.stack([res.results[b]["out"] for b in range(N_CORES)], axis=0)
    return out, res


def kernel(x):
    out, _ = run_spmd(x)
    return out
